# revision 9
# baseline (speedup 1.0000x reference)
"""AliasFreeActivation Trainium2 kernel (v5: Prelu evac, const-weight s4).

out = crop10(down2(leaky_relu(up4(x + bias)) * sqrt2))   [4,256,236,236]

Per (batch,channel) image (1024 images, 128 per core), with A the up4
matrix [128,512] and D the down2 matrix [512,256] (both banded):

  s1  v1[w,ho]  = sum_h xb[h,w] A[h,ho]              1 MM N=512
  s2  Y[ho,wo]  = sum_w v1[w,ho] (sqrt2*A)[w,wo]     4 MM N=512
  act L = prelu(Y, 0.2)     (= sqrt2*leaky_relu(up4(xb)); scale folded
                             into s2's matrix — ACT Prelu honors alpha)
  s3  z[wo,hd]  = sum_ho L[ho,wo] D[ho,hd]          16 MM banded N<=70
  s4  oT[wd,hd] = sum_wo D[wo,wd] z[wo,hd]           5 MM N=236
      (s4 keeps the CONSTANT D stationary — weight loads don't scale
       with image data; output comes out transposed, fixed on host)

Weight-load (LDWEIGHTS) is the serial bottleneck on the PE: this layout
streams 26 weight loads/image (vs 32 in v3) and drops the whole linear
side path.  Evacuations are balanced: ACT runs the two fused Prelu
passes over the fine grid, DVE does the V1/z/out casts, GPSIMD adds the
bias.  I/O is fp16 both ways (host casts).
"""
import numpy as np

UP, DOWN, MARGIN, NEG_SLOPE = 4, 2, 10, 0.2
SQRT2 = 1.4142135623730951
H = W = 128
OUT = 236
NCORES = 8
NIMG = 128

# down-matrix window per 128-row K-chunk: D[s,o] nonzero for o in [64k-3, 64k+66]
DWIN = [(0, 67), (61, 131), (125, 195), (189, 256)]
# s4 blocks (chunk k, out-group g) with g0 = wd 10..137, g1 = wd 138..265
S4MM = [(0, 0), (1, 0), (2, 0), (2, 1), (3, 1)]

CM_A = 0
CM_A2 = 512
CM_DW = 1024          # 4 windows, 70 cols apart
CM_DH = 1304          # 5 dense [128,128] blocks
CM_SA = 1944          # column sums of A, replicated across partitions
CM_COLS = 2456

_cache = {}


def _build_nc(nimg=NIMG):
    import concourse.bacc as bacc
    import concourse.bass as bass
    import concourse.tile as tile
    from concourse import mybir

    F32 = mybir.dt.float32
    F16 = mybir.dt.float16
    AF = mybir.ActivationFunctionType
    ALU = mybir.AluOpType

    nc = bacc.Bacc("TRN2", target_bir_lowering=False)
    x_d = nc.dram_tensor("x", [nimg, H, W], F16, kind="ExternalInput")
    b_d = nc.dram_tensor("bias", [nimg], F32, kind="ExternalInput")
    c_d = nc.dram_tensor("cm", [128, CM_COLS], F16, kind="ExternalInput")
    sa_d = nc.dram_tensor("sa", [128, 512], F16, kind="ExternalInput")
    o_d = nc.dram_tensor("out", [nimg, 2, 128, OUT], F16, kind="ExternalOutput")

    with tile.TileContext(nc) as tc:
        with (
            tc.tile_pool(name="const", bufs=1) as const,
            tc.tile_pool(name="xin", bufs=6) as xin,
            tc.tile_pool(name="v1p", bufs=2) as v1p,
            tc.tile_pool(name="yp", bufs=2) as yp,
            tc.tile_pool(name="zp", bufs=2) as zp,
            tc.tile_pool(name="ofp", bufs=4) as ofp,
            tc.tile_pool(name="p1", bufs=1, space="PSUM") as p1p,
            tc.tile_pool(name="p2", bufs=2, space="PSUM") as p2p,
            tc.tile_pool(name="p34", bufs=1, space="PSUM") as p34p,
            tc.tile_pool(name="pt", bufs=1, space="PSUM") as ptp,
        ):
            cm = const.tile([128, CM_COLS], F16)
            nc.sync.dma_start(out=cm, in_=c_d[:])
            A_sb = cm[:, CM_A:CM_A + 512]
            A2_sb = cm[:, CM_A2:CM_A2 + 512]
            SA_sb = const.tile([128, 512], F16)
            nc.sync.dma_start(out=SA_sb, in_=sa_d[:])

            def D_sb(k):
                o0, o1 = DWIN[k]
                return cm[:, CM_DW + 70 * k: CM_DW + 70 * k + (o1 - o0)]

            def Dh_sb(j):
                return cm[:, CM_DH + 128 * j: CM_DH + 128 * (j + 1)]

            bb = const.tile([128, nimg], F32)
            nc.gpsimd.dma_start(
                out=bb,
                in_=bass.AP(tensor=b_d[:].tensor, offset=0,
                            ap=[[0, 128], [1, nimg]]),
            )

            # warm PE's clock on the const DMA lane
            pwarm = p2p.tile([128, 2, 512], F32, name="p2")
            nc.tensor.matmul(out=pwarm[:32, 0, :256], lhsT=cm[:, :32],
                             rhs=cm[:, :256], start=True, stop=True)

            def s1_mm(i):
                # s1: up vertical (bias folds in during evacuation)
                X = xin.tile([128, W], F16)
                nc.sync.dma_start(out=X, in_=x_d[i])
                P1 = p1p.tile([128, 512], F32)
                nc.tensor.matmul(out=P1, lhsT=X, rhs=A_sb,
                                 start=True, stop=True)
                return P1

            def s1_evac(i, P1):
                # v1 = A^T(x + b) = A^T x + b * colsum(A)
                V1 = v1p.tile([128, 512], F16)
                nc.vector.scalar_tensor_tensor(
                    out=V1, in0=SA_sb, scalar=bb[:, i:i + 1], in1=P1,
                    op0=ALU.mult, op1=ALU.add)
                return V1

            def of_evac(ip, PT):
                # deferred one iteration: deps are long done, so these
                # never block the engine FIFOs
                OF = ofp.tile([128, 2, OUT], F16)
                nc.vector.tensor_copy(out=OF[:, 0, :], in_=PT[:, 0, :])
                nc.scalar.copy(out=OF[0:OUT - 128, 1, :],
                               in_=PT[0:OUT - 128, 1, :])
                nc.sync.dma_start(
                    out=bass.AP(tensor=o_d[:].tensor,
                                offset=ip * 2 * 128 * OUT,
                                ap=[[OUT, 128], [128 * OUT, 2], [1, OUT]]),
                    in_=OF)

            for i in range(nimg):
                V1 = s1_evac(i, s1_mm(i))

                # s2 + fused leaky-relu evacuation (fine grid)
                Y = yp.tile([128, 4, 512], F16)
                for pr in range(2):
                    P2 = p2p.tile([128, 2, 512], F32, name="p2")
                    for h in range(2):
                        m = 2 * pr + h
                        nc.tensor.matmul(out=P2[:, h, :],
                                         lhsT=V1[:, 128 * m:128 * (m + 1)],
                                         rhs=A2_sb, start=True, stop=True)
                    nc.scalar.activation(out=Y[:, 2 * pr:2 * pr + 2, :],
                                         in_=P2, func=AF.Prelu,
                                         bias=0.0, scale=1.0, alpha=NEG_SLOPE)


                if i > 0:
                    of_evac(*PTlag)
                PTlag = None

                # s3: down vertical (banded), all four wo-chunks in one
                # 2-bank PSUM tile, single evacuation
                P34 = p34p.tile([128, 4, 256], F32)
                for m in range(4):
                    for k in range(4):
                        o0, o1 = DWIN[k]
                        nc.tensor.matmul(
                            out=P34[:, m, o0:o1],
                            lhsT=Y[:, k, 128 * m:128 * (m + 1)],
                            rhs=D_sb(k), start=(k == 0), stop=(k == 3))
                Z = zp.tile([128, 4, 256], F16)
                nc.vector.tensor_copy(out=Z, in_=P34)

                # s4: down horizontal with D stationary -> transposed out
                PT = ptp.tile([128, 2, OUT], F32)
                for j, (k, g) in enumerate(S4MM):
                    nc.tensor.matmul(out=PT[:, g, :], lhsT=Dh_sb(j),
                                     rhs=Z[:, k, MARGIN:MARGIN + OUT],
                                     start=(j == 0), stop=(j == len(S4MM) - 1))
                PTlag = (i, PT)

            of_evac(*PTlag)

    nc.finalize()
    return nc


def _filter_matrices(up_filter, down_filter):
    fu = np.asarray(up_filter, dtype=np.float64)
    fd = np.asarray(down_filter, dtype=np.float64)
    i = np.arange(128)[:, None]
    o = np.arange(512)[None, :]
    t = 10 + o - 4 * i
    A = np.where((t >= 0) & (t < 24), fu[np.clip(t, 0, 23)], 0.0)
    s = np.arange(512)[:, None]
    o2 = np.arange(256)[None, :]
    t2 = 6 + 2 * o2 - s
    D = np.where((t2 >= 0) & (t2 < 12), fd[np.clip(t2, 0, 11)], 0.0)
    return A, D


def _pack_consts(up_filter, down_filter):
    A, D = _filter_matrices(up_filter, down_filter)
    cm = np.zeros((128, CM_COLS), dtype=np.float16)
    cm[:, CM_A:CM_A + 512] = A.astype(np.float16)
    cm[:, CM_A2:CM_A2 + 512] = (A * SQRT2).astype(np.float16)
    for k, (o0, o1) in enumerate(DWIN):
        cm[:, CM_DW + 70 * k: CM_DW + 70 * k + (o1 - o0)] = \
            D[128 * k:128 * (k + 1), o0:o1].astype(np.float16)
    Dpad = np.concatenate([D, np.zeros((512, 10))], axis=1)
    for j, (k, g) in enumerate(S4MM):
        c0 = MARGIN + 128 * g
        cm[:, CM_DH + 128 * j: CM_DH + 128 * (j + 1)] = \
            Dpad[128 * k:128 * (k + 1), c0:c0 + 128].astype(np.float16)
    cm[:, CM_SA:CM_SA + 512] = np.tile(A.sum(axis=0, keepdims=True),
                                       (128, 1)).astype(np.float16)
    return cm


def _run(x, bias, up_filter, down_filter, trace=False):
    from concourse.bass_utils import run_bass_kernel_spmd

    if "nc" not in _cache:
        _cache["nc"] = _build_nc()
    nc = _cache["nc"]

    cm = _pack_consts(up_filter, down_filter)
    xf = np.ascontiguousarray(np.asarray(x).astype(np.float16)
                              .reshape(NCORES * NIMG, H, W))
    bias = np.asarray(bias, dtype=np.float32)
    bias_full = np.tile(bias, (NCORES * NIMG) // bias.shape[0])

    in_maps = []
    for c in range(NCORES):
        in_maps.append({
            "x": xf[NIMG * c: NIMG * (c + 1)],
            "bias": np.ascontiguousarray(bias_full[NIMG * c: NIMG * (c + 1)]),
            "cm": cm,
            "sa": np.ascontiguousarray(cm[:, CM_SA:CM_SA + 512]),
        })
    res = run_bass_kernel_spmd(nc, in_maps, core_ids=list(range(NCORES)),
                               trace=trace)
    out = np.concatenate([res.results[c]["out"][None] for c in range(NCORES)], 0)
    out = out.reshape(NCORES * NIMG, 2, 128, OUT)
    out = np.concatenate([out[:, 0, :, :], out[:, 1, 0:OUT - 128, :]], axis=1)
    out = out.reshape(4, 256, OUT, OUT)
    # device produced [wd, hd]; reference wants [hd, wd]
    out = np.ascontiguousarray(out.swapaxes(2, 3)).astype(np.float32)
    return out, res


def kernel(x, bias, up_filter, down_filter):
    out, _ = _run(x, bias, up_filter, down_filter, trace=False)
    return out


def kernel_traced(x, bias, up_filter, down_filter):
    return _run(x, bias, up_filter, down_filter, trace=True)
